# revision 6
# baseline (speedup 1.0000x reference)
"""DAG additive-attention kernel for 8 Trainium2 NeuronCores.

Reference computation (single fp32 graph):
    x = concat([leaves, ancestors], -1)            # [N, 2048]
    h = tanh(x @ W1 + b1)                          # [N, 512]
    scores = h @ W2 + b2                           # [N, 1]
    attn = softmax(scores, axis=0)
    out = attn.squeeze(1) @ ancestors              # [1024]

Distribution: shard N across the 8 cores. Softmax over N needs no on-device
collective: b2 is a constant shift (softmax-invariant, dropped), scores are
bounded, so exp() without max subtraction is safe in fp32. Each core returns
(sum_i exp(s_i) * ancestors_i, sum_i exp(s_i)) over its shard and the host
combines in fp64.

The x @ W1 contraction runs in fp8(e4m3) DoubleRow matmuls (2 fp8 MACs per PE
cell per cycle): x is pre-scaled by 16 and W1 by 1024 on the host, both
clipped inside TRN e4m3's +-240 range. Plain RNE quantization costs ~2% on
the output (the score error does not average out across the softmax), so the
host uses projection-compensated rounding: W1 rows are rounded so the
rounding error is orthogonal to W2, and x rows are rounded so the error is
orthogonal to v = W1q @ W2. This zeroes the first-order score error (the
constant-tanh-slope term) on both operands and lands the end-to-end error at
~2.8e-3 while keeping the full DoubleRow rate (one 256-contraction matmul per
216 ns).

Per core (R = 8192 rows, 16 slabs of 512): the main loop is a pure DR matmul
stream (8 per 128-row tile) with a fused DVE descale+bias, ACT tanh, fused
DVE w2-mult+row-reduce, and ACT exp into a [128, 64] grid of softmax
numerators. All 16 ancestor slabs stay resident in SBUF, and the step-3
weighted-sum matmuls (bf16) run as one batch at the end -- gated behind a
tiny copy of the weight grid -- so the DR stream never pays DoubleRow<->bf16
mode transitions. The PE clock gate is pre-warmed with dummy matmuls during
the initial DMA window.
"""

import sys

for _p in ("/opt/trn_rl_repo", "/opt/pypackages"):
    if _p not in sys.path:
        sys.path.append(_p)

from concurrent.futures import ThreadPoolExecutor
from contextlib import ExitStack

import ml_dtypes
import numpy as np

import concourse.tile as tile
from concourse import bacc, mybir
from concourse.bass import ts
from concourse.bass_utils import run_bass_kernel_spmd

N, EMB, ATT = 65536, 1024, 512
N_CORES = 8
R = N // N_CORES          # rows per core
KF = 2 * EMB              # contraction (feature) dim
KC = KF // 128            # k-chunks of 128
KD = KC // 2              # fp8 DoubleRow double-chunks of 256
SLAB = 512                # rows per DMA slab
NSLAB = R // SLAB
SUB = SLAB // 128         # 128-row subtiles per slab
NT = R // 128             # row tiles per core
BF16 = mybir.dt.bfloat16
F32 = mybir.dt.float32
F8 = mybir.dt.float8e4

X_SCALE = 16.0            # x pre-scale before fp8 quantization
W1_SCALE = 1024.0         # W1 pre-scale before fp8 quantization
DESCALE = 1.0 / (X_SCALE * W1_SCALE)
FP8_CLIP = 224.0          # stay inside TRN e4m3 max normal (+-240)

LAST_RESULTS = None       # BassKernelResults of the most recent run

WARM_MMS = 52  # PE clock-gate warmup matmuls emitted before the main loop

_f8 = ml_dtypes.float8_e4m3
_bf = ml_dtypes.bfloat16


def _kernel_body(ctx, tc, xT, anc, w1, b1bc, w2b, partial_d, wx_out_d):
    nc = tc.nc
    singles = ctx.enter_context(tc.tile_pool(name="singles", bufs=1))
    xt_pool = ctx.enter_context(tc.tile_pool(name="xt", bufs=4))
    anc_pool = ctx.enter_context(tc.tile_pool(name="anc", bufs=NSLAB))
    hb_pool = ctx.enter_context(tc.tile_pool(name="hb", bufs=4))
    th_pool = ctx.enter_context(tc.tile_pool(name="th", bufs=3))
    sc_pool = ctx.enter_context(tc.tile_pool(name="sc", bufs=4))
    h_pool = ctx.enter_context(tc.tile_pool(name="hps", bufs=4, space="PSUM"))
    acc_pool = ctx.enter_context(tc.tile_pool(name="acc", bufs=1, space="PSUM"))
    out_pool = ctx.enter_context(tc.tile_pool(name="outs", bufs=1))

    # Weights / constants, resident for the whole kernel, issued on the
    # scalar-engine HWDGE ring so they overlap the first xT slab (sync ring).
    # All DRAM operands arrive pre-tiled from the host so every transfer
    # below is a fully contiguous read.
    w1_sb = singles.tile([128, KC, ATT], F8)
    for q in range(4):
        nc.scalar.dma_start(w1_sb[:, ts(q, KC // 4), :], w1[:, ts(q, KC // 4), :])
    b1bc_sb = singles.tile([128, ATT], BF16)
    nc.scalar.dma_start(b1bc_sb[:], b1bc[:])
    w2b_sb = singles.tile([128, ATT], BF16)
    nc.scalar.dma_start(w2b_sb[:], w2b[:])
    ones_mv = singles.tile([128, 1], BF16)
    nc.vector.memset(ones_mv[:], 1.0)

    # Warm the PE clock gate during the initial DMA window: dependency-free
    # matmuls on a zeroed tile keep TensorE busy >3.4us so the HAM throttle is
    # released right around the time the first slab and weights arrive.
    if WARM_MMS:
        warm_sb = singles.tile([128, 128], BF16)
        nc.vector.memset(warm_sb[:], 0.0)
        warm_ps = h_pool.tile([128, 128], F32, tag="h")
        for _ in range(WARM_MMS):
            nc.tensor.matmul(
                warm_ps[:], warm_sb[:], warm_sb[:], start=True, stop=True
            )

    # Softmax numerators for all 64 row tiles, written column-by-column by
    # the ACT exp; later copied into wxA/wxB which gate the end-batched
    # step-3 matmul burst (the copies keep the scheduler from interleaving
    # bf16 step-3 matmuls into the DoubleRow stream).
    wxall = singles.tile([128, NT], BF16)
    wxA = singles.tile([128, NT - SUB], BF16)
    wxB = singles.tile([128, SUB], BF16)

    # Persistent PSUM accumulators: weighted ancestor sums (acc0/acc1) and
    # the per-tile sums of the exp weights (acc_se, reduced on host).
    acc0 = acc_pool.tile([1, ATT], F32, tag="acc0")
    acc1 = acc_pool.tile([1, ATT], F32, tag="acc1")
    acc_se = acc_pool.tile([NT, 1], F32, tag="accse")

    def emit_score_tail(prev_th, prev_t):
        """w2-mult + row-reduce + exp for a tile whose tanh already ran.

        Emitted one tile late so the in-order DVE queue never stalls
        waiting on the ACT tanh -- DVE stays ahead of the PE and the h
        PSUM buffers free early."""
        sc = sc_pool.tile([128, 1], F32, tag="sc")
        thw = hb_pool.tile([128, ATT], BF16, tag="hb")
        nc.vector.scalar_tensor_tensor(
            thw[:], prev_th[:], 1.0, w2b_sb[:],
            mybir.AluOpType.bypass, mybir.AluOpType.mult,
            accum_out=sc[:],
        )
        nc.scalar.activation(wxall[:, prev_t : prev_t + 1], sc[:],
                             mybir.ActivationFunctionType.Exp)

    an_tiles = []
    prev = None
    for s in range(NSLAB):
        xt = xt_pool.tile([128, KC, SLAB], F8, tag="xt")
        if s == 0:
            # Split the first slab by k-chunks (contiguous in the tiled
            # layout) so the opening k-loop can start after the first
            # quarter arrives, pipelining through the rest.
            for q in range(4):
                nc.sync.dma_start(
                    xt[:, ts(q, KC // 4), :], xT[s, :, ts(q, KC // 4), :]
                )
        else:
            nc.sync.dma_start(xt[:], xT[s])
        # Ancestors ride the idle GpSimd HWDGE ring: the sync ring alone
        # cannot sustain xt + anc at the DR-stream consumption rate.
        an = anc_pool.tile([128, SUB, EMB], BF16, tag="an")
        an_tiles.append(an)
        nc.gpsimd.dma_start(an[:], anc[s])
        for u in range(SUB):
            t = s * SUB + u
            if t == NT - SUB + 1:
                # Gate for the step-3 burst over tiles 0..NT-SUB-1; emitted
                # here (not earlier) so the DVE-queue position sits after
                # the score ops it depends on.
                nc.vector.tensor_copy(wxA[:], wxall[:, 0 : NT - SUB])
            h = h_pool.tile([128, ATT], F32, tag="h")
            for c in range(KD):
                nc.tensor.matmul(
                    h[:], xt[:, ts(c, 2), ts(u, 128)], w1_sb[:, ts(c, 2), :],
                    start=(c == 0), stop=(c == KD - 1),
                    perf_mode=mybir.MatmulPerfMode.DoubleRow,
                )
            hb = hb_pool.tile([128, ATT], BF16, tag="hb")
            nc.vector.scalar_tensor_tensor(
                hb[:], h[:], DESCALE, b1bc_sb[:],
                mybir.AluOpType.mult, mybir.AluOpType.add,
            )
            th = th_pool.tile([128, ATT], BF16, tag="th")
            nc.scalar.activation(th[:], hb[:], mybir.ActivationFunctionType.Tanh)
            if prev is not None:
                emit_score_tail(*prev)
            prev = (th, t)

    emit_score_tail(*prev)
    nc.vector.tensor_copy(wxB[:], wxall[:, NT - SUB : NT])

    # End-batched step-3: weighted ancestor sums over all tiles. Tiles of the
    # final slab go through wxB so the burst for earlier tiles can run while
    # the last score chains finish.
    def step3(t, wx_sb, col):
        s, u = divmod(t, SUB)
        st, sp = (t == 0), (t == NT - 1)
        an_t = an_tiles[s]
        nc.tensor.matmul(acc0[:], wx_sb[:, col : col + 1], an_t[:, u, 0:ATT],
                         start=st, stop=sp)
        nc.tensor.matmul(acc1[:], wx_sb[:, col : col + 1], an_t[:, u, ATT:EMB],
                         start=st, stop=sp)

    for t in range(NT - SUB):
        step3(t, wxA, t)
    for t in range(NT - SUB, NT):
        step3(t, wxB, t - (NT - SUB))
    nc.tensor.matmul(acc_se[:], wxall[:], ones_mv[:], start=True, stop=True)

    out_sb = out_pool.tile([1, EMB], F32)
    se_sb = out_pool.tile([NT, 1], F32)
    nc.vector.tensor_copy(out_sb[:, 0:ATT], acc0[:])
    nc.scalar.activation(out_sb[:, ATT:EMB], acc1[:],
                         mybir.ActivationFunctionType.Copy)
    nc.vector.tensor_copy(se_sb[:], acc_se[:])
    nc.sync.dma_start(partial_d[:], out_sb[:])
    nc.scalar.dma_start(wx_out_d[:], se_sb[:])


_nc_cache = None


def _get_nc():
    global _nc_cache
    if _nc_cache is None:
        nc = bacc.Bacc(
            "TRN2", target_bir_lowering=False, debug=False, num_devices=N_CORES
        )
        xT = nc.dram_tensor(
            "xT", [NSLAB, 128, KC, SLAB], F8, kind="ExternalInput"
        ).ap()
        anc = nc.dram_tensor(
            "anc", [NSLAB, 128, SUB, EMB], BF16, kind="ExternalInput"
        ).ap()
        w1 = nc.dram_tensor("w1", [128, KC, ATT], F8, kind="ExternalInput").ap()
        b1bc = nc.dram_tensor("b1bc", [128, ATT], BF16, kind="ExternalInput").ap()
        w2b = nc.dram_tensor("w2b", [128, ATT], BF16, kind="ExternalInput").ap()
        partial = nc.dram_tensor("partial", [1, EMB], F32, kind="ExternalOutput").ap()
        wx_out = nc.dram_tensor("wx_out", [NT, 1], F32, kind="ExternalOutput").ap()
        with tile.TileContext(nc) as tc, ExitStack() as ctx:
            _kernel_body(ctx, tc, xT, anc, w1, b1bc, w2b, partial, wx_out)
        nc.compile()
        _nc_cache = nc
    return _nc_cache


def _rne8(a):
    return np.clip(a, -FP8_CLIP, FP8_CLIP).astype(_f8).astype(np.float32)


def _fb_round_w1(w1_scaled, proj):
    """Round each row of w1_scaled to fp8 with the rounding error projected
    out of `proj` (greedy per-column pass, vectorized over rows)."""
    q = _rne8(w1_scaled)
    err = q - w1_scaled
    r = err @ proj
    ulp = np.abs(np.spacing(q.astype(_f8)).astype(np.float32))
    ulp = np.maximum(ulp, 2.0 ** -9)
    alt = np.where(err > 0, q - ulp, q + ulp)
    alt = alt.astype(_f8).astype(np.float32)
    dproj = (alt - q) * proj[None, :]
    cost = np.abs(alt - w1_scaled) - np.abs(err)
    for j in range(w1_scaled.shape[1]):
        dj = dproj[:, j]
        flip = (np.abs(r + dj) < np.abs(r)) & (cost[:, j] < 1.5 * ulp[:, j])
        r = np.where(flip, r + dj, r)
        q[:, j] = np.where(flip, alt[:, j], q[:, j])
    return q


def _fb_round_x(xs, proj, gate=0.6):
    """Projection-compensated fp8 rounding for a slab of x rows: RNE, then
    flip a prefix of low-cost sign-opposing entries (by cumulative |dproj|)
    so each row's rounding error is ~orthogonal to proj. Fully vectorized."""
    q = _rne8(xs)
    err = q - xs
    r = err @ proj
    ulp = np.abs(np.spacing(q.astype(_f8)).astype(np.float32))
    ulp = np.maximum(ulp, 2.0 ** -9)
    alt = np.where(err > 0, q - ulp, q + ulp)
    alt = alt.astype(_f8).astype(np.float32)
    dproj = (alt - q) * proj[None, :]
    cost = np.abs(alt - xs) - np.abs(err)
    s = np.sign(r)[:, None]
    cand = (dproj * s < 0) & (cost < gate * ulp)
    step = np.abs(dproj) * cand
    csum = np.cumsum(step, axis=1)
    need = np.abs(r)[:, None]
    reached = csum >= need
    first = np.argmax(reached, axis=1)
    first = np.where(~reached[:, -1], xs.shape[1] - 1, first)
    idx = np.arange(xs.shape[1])[None, :]
    flip = cand & (idx <= first[:, None])
    return np.where(flip, alt, q)


def _prep_core(c, leaves, ancestors, proj, shared):
    EC = EMB // 128  # feature chunks per source tensor
    xT = np.empty((NSLAB, 128, KC, SLAB), dtype=_f8)
    at = np.empty((NSLAB, 128, SUB, EMB), dtype=_bf)
    xs = np.empty((SLAB, KF), dtype=np.float32)
    for s in range(NSLAB):
        rs = slice(c * R + s * SLAB, c * R + (s + 1) * SLAB)
        av = ancestors[rs]
        np.multiply(leaves[rs], X_SCALE, out=xs[:, 0:EMB])
        np.multiply(av, X_SCALE, out=xs[:, EMB:KF])
        q = _fb_round_x(xs, proj)
        np.copyto(
            xT[s, :, 0:EC, :], q[:, 0:EMB].reshape(SLAB, EC, 128).transpose(2, 1, 0),
            casting="unsafe",
        )
        np.copyto(
            xT[s, :, EC:KC, :],
            q[:, EMB:KF].reshape(SLAB, EC, 128).transpose(2, 1, 0),
            casting="unsafe",
        )
        np.copyto(
            at[s], av.reshape(SUB, 128, EMB).transpose(1, 0, 2), casting="unsafe"
        )
    return {"xT": xT, "anc": at, **shared}


def kernel(leaves, ancestors, W1, b1, W2, b2, *, trace=False):
    global LAST_RESULTS
    nc = _get_nc()
    leaves = np.asarray(leaves, dtype=np.float32)
    ancestors = np.asarray(ancestors, dtype=np.float32)
    W1 = np.asarray(W1, dtype=np.float32)
    w2f = np.asarray(W2, dtype=np.float32).reshape(ATT)
    w1q = _fb_round_w1(W1 * W1_SCALE, w2f)          # [2048, 512] scaled, on-grid
    proj = (w1q @ w2f) * (1.0 / W1_SCALE)           # v = W1q @ W2, unscaled
    shared = {
        "w1": np.ascontiguousarray(
            w1q.reshape(KC, 128, ATT).transpose(1, 0, 2).astype(_f8)
        ),
        "b1bc": np.ascontiguousarray(
            np.broadcast_to(np.asarray(b1).astype(_bf).reshape(1, ATT), (128, ATT))
        ),
        "w2b": np.ascontiguousarray(
            np.broadcast_to(w2f.astype(_bf).reshape(1, ATT), (128, ATT))
        ),
    }
    with ThreadPoolExecutor(max_workers=8) as ex:
        in_maps = list(
            ex.map(
                lambda c: _prep_core(c, leaves, ancestors, proj, shared),
                range(N_CORES),
            )
        )
    res = run_bass_kernel_spmd(
        nc, in_maps, core_ids=list(range(N_CORES)), trace=trace
    )
    LAST_RESULTS = res
    num = np.zeros(EMB, dtype=np.float64)
    den = 0.0
    for c in range(N_CORES):
        num += res.results[c]["partial"][0].astype(np.float64)
        den += res.results[c]["wx_out"].astype(np.float64).sum()
    return (num / den).astype(np.float32)
